# revision 1
# baseline (speedup 1.0000x reference)
"""Trainium2 Bass kernel for the MiniBatchAUC pairwise surrogate loss.

Math: with s = sigmoid(logits), pos/neg the 0/1 target masks,
    loss_sum = sum_{i in P, j in N} (1 - s_i + s_j)^2
factorizes exactly (expand the square; the double sum separates):
    loss_sum = n_neg * Sp2 + 2 * Sp1 * Sn1 + n_pos * Sn2
      Sp1 = sum_P (1-s),  Sp2 = sum_P (1-s)^2,
      Sn1 = sum_N s,      Sn2 = sum_N s^2,
and with c = sum T, m1 = sum T*s, m2 = sum T*s^2, g1 = sum s, g2 = sum s^2:
      Sp1 = c - m1, Sp2 = c - 2*m1 + m2, Sn1 = g1 - m1, Sn2 = g2 - m2.
So the O(N^2) pairwise matrix is never materialized: each core reduces its
2048-element shard to 5 per-partition partial sums; the host all-reduces
the per-core partials and applies the closed form.

Per-core device program (SPMD, identical on all 8 cores):
  - one DMA in: [128, 32] f32 tile = logits(16 cols) | targets(16)
  - ACT: s = sigmoid(L) (fused accum -> per-partition sum s),
         count = Copy(T) (fused accum -> per-partition sum T)
  - DVE: s*s, T*s, (T*s)*s multiplies + reduce_sum of each
    (tensor_tensor_reduce crashes this terminal's runtime; ACT Square in the
     s -> s2 chain is slower than overlapping the multiply on DVE)
  - one DMA out: the [128, 5] per-partition partials (2.5 KB)
No PE/PSUM involvement - the partition reduction is part of the host-side
all-reduce of partials (TimelineSim: 6794 ns vs 7537 ns with an
on-device ones-matmul partition reduction).

Written in raw bacc (manual semaphores, no TileContext) so the program
carries no Tile exit drain / EVSEM butterfly: 6589 ns modeled vs 6794 ns
for the identical Tile-scheduled program, and the real-hardware tail cost
of the Tile barrier is documented as multi-microsecond. Same-engine RAW
hazards are semaphore-chained (deep pipelines reorder retirement); the
schedule was validated race-free in CoreSim and bit-exact on hardware.
"""

import numpy as np

try:
    import concourse.bass as bass
except ImportError:  # concourse ships in the container, not on sys.path
    import sys

    sys.path.insert(0, "/opt/trn_rl_repo")
    import concourse.bass as bass

import concourse.tile as tile
from concourse import bacc, mybir
from concourse import bass_utils

N = 16384
NCORES = 8
SHARD = N // NCORES  # 2048 elements per core
P = 128  # SBUF partitions
F = SHARD // P  # 16 free elements per partition

f32 = mybir.dt.float32

_CACHE: dict = {}


def _build():
    nc = bacc.Bacc(
        "TRN2",
        target_bir_lowering=False,
        debug=False,
        enable_asserts=False,
        num_devices=NCORES,
    )
    x_dram = nc.dram_tensor("x", [P, 2 * F], f32, kind="ExternalInput").ap()
    o_dram = nc.dram_tensor("o", [P, 5], f32, kind="ExternalOutput").ap()

    Sig = mybir.ActivationFunctionType.Sigmoid
    Copy = mybir.ActivationFunctionType.Copy
    X = mybir.AxisListType.X

    # Raw bacc with manual semaphores: no TileContext, so the Tile exit
    # drain + EVSEM butterfly never enters the program.
    with (
        nc.sbuf_tensor([P, 2 * F], f32) as x,
        nc.sbuf_tensor([P, F], f32) as s,
        nc.sbuf_tensor([P, F], f32) as s2,
        nc.sbuf_tensor([P, F], f32) as tcnt,
        nc.sbuf_tensor([P, F], f32) as ts,
        nc.sbuf_tensor([P, F], f32) as ts2,
        nc.sbuf_tensor([P, 5], f32) as r,  # g1 | g2 | c | m1 | m2
        nc.semaphore() as dsem,
        nc.semaphore() as asem,
        nc.semaphore() as vsem,
        nc.semaphore() as osem,
        nc.Block() as block,
    ):
        L = x[:, 0:F]
        T = x[:, F : 2 * F]

        @block.sync
        def _(sync):
            sync.dma_start(x[:], x_dram).then_inc(dsem, 16)
            sync.wait_ge(asem, 2)  # both ACT accums landed in r
            sync.wait_ge(vsem, 6)  # all DVE muls + reduces landed in r
            sync.dma_start(o_dram, r[:]).then_inc(osem, 16)
            sync.wait_ge(osem, 16)  # out-DMA complete before program end

        @block.scalar
        def _(scalar):
            scalar.wait_ge(dsem, 16)
            nc.scalar.activation(s[:], L, Sig, accum_out=r[:, 0:1]).then_inc(asem, 1)
            nc.scalar.activation(tcnt[:], T, Copy, accum_out=r[:, 2:3]).then_inc(
                asem, 1
            )

        @block.vector
        def _(vector):
            # Deep engine pipelines: same-engine RAW hazards need sem chains
            # (the race detector rejects back-to-back dependent DVE ops).
            vector.wait_ge(dsem, 16)  # T in SBUF
            vector.wait_ge(asem, 1)  # s written
            nc.vector.tensor_mul(ts[:], T, s[:]).then_inc(vsem, 1)
            nc.vector.tensor_mul(s2[:], s[:], s[:]).then_inc(vsem, 1)
            vector.wait_ge(vsem, 1)  # ts retired
            nc.vector.tensor_mul(ts2[:], ts[:], s[:]).then_inc(vsem, 1)
            nc.vector.reduce_sum(r[:, 3:4], ts[:], axis=X).then_inc(vsem, 1)
            vector.wait_ge(vsem, 2)  # s2 retired
            nc.vector.reduce_sum(r[:, 1:2], s2[:], axis=X).then_inc(vsem, 1)
            vector.wait_ge(vsem, 3)  # ts2 retired
            nc.vector.reduce_sum(r[:, 4:5], ts2[:], axis=X).then_inc(vsem, 1)

    nc.compile()
    return nc


def _get_nc():
    if "nc" not in _CACHE:
        _CACHE["nc"] = _build()
    return _CACHE["nc"]


def make_in_maps(logits: np.ndarray, targets: np.ndarray) -> list[dict]:
    logits = np.ascontiguousarray(logits, dtype=np.float32)
    t32 = np.asarray(targets).astype(np.float32)  # values are 0/1; lossless
    in_maps = []
    for k in range(NCORES):
        sl = slice(k * SHARD, (k + 1) * SHARD)
        xk = np.empty((P, 2 * F), np.float32)
        xk[:, 0:F] = logits[sl].reshape(P, F)
        xk[:, F : 2 * F] = t32[sl].reshape(P, F)
        in_maps.append({"x": xk})
    return in_maps


def combine(outs: np.ndarray) -> np.ndarray:
    """All-reduce the [NCORES, P, 5] partials and apply the closed form."""
    tot = outs.astype(np.float64).sum(axis=(0, 1))
    g1, g2, c, m1, m2 = tot
    n_pos = c
    n_neg = float(N) - c
    sp1 = c - m1
    sp2 = c - 2.0 * m1 + m2
    sn1 = g1 - m1
    sn2 = g2 - m2
    loss = (n_neg * sp2 + 2.0 * sp1 * sn1 + n_pos * sn2) / (n_pos * n_neg)
    return np.array(loss, dtype=np.float32)


def kernel(logits: np.ndarray, targets: np.ndarray, **run_kwargs):
    nc = _get_nc()
    res = bass_utils.run_bass_kernel_spmd(
        nc, make_in_maps(logits, targets), core_ids=list(range(NCORES)), **run_kwargs
    )
    outs = np.stack([r["o"] for r in res.results])  # [8, 128, 5]
    out = combine(outs)
    _CACHE["last_results"] = res
    return out



# revision 2
# speedup vs baseline: 1.4037x; 1.4037x over previous
"""Trainium2 Bass kernel for the MiniBatchAUC pairwise surrogate loss.

Math: with s = sigmoid(logits), pos/neg the 0/1 target masks,
    loss_sum = sum_{i in P, j in N} (1 - s_i + s_j)^2
factorizes exactly (expand the square; the double sum separates):
    loss_sum = n_neg * Sp2 + 2 * Sp1 * Sn1 + n_pos * Sn2
      Sp1 = sum_P (1-s),  Sp2 = sum_P (1-s)^2,
      Sn1 = sum_N s,      Sn2 = sum_N s^2,
and with c = sum T, m1 = sum T*s, m2 = sum T*s^2, g1 = sum s, g2 = sum s^2:
      Sp1 = c - m1, Sp2 = c - 2*m1 + m2, Sn1 = g1 - m1, Sn2 = g2 - m2.
So the O(N^2) pairwise matrix is never materialized: each core reduces its
2048-element shard to 5 per-partition partial sums; the host all-reduces
the per-core partials and applies the closed form.

Per-core device program (SPMD, identical on all 8 cores), raw bacc with
manual semaphores. Schedule built around the three fixed DMA latencies the
cost model charges (HWDGE descriptor gen ~625ns, DGE->DMA delay ~650ns,
DMA-completion->semaphore ~900ns):

  SP   : in-DMA [128,32] issued at t~25 (entry barrier stripped, see below);
         zero-DMA clearing o[:,0:5] (output DRAM is NOT zero-initialized -
         verified empirically - and the output path is a scatter-ADD);
         final wait on the scatter-DMA completion sem.
  ACT  : one sigmoid over the 16 logit columns, no accum_out (the
         accumulator read costs an extra 187ns slice and delays the sem all
         consumers wait on; reductions are cheaper on DVE).
  DVE  : all eight 77ns ops: c = reduce(T) runs inside the sigmoid's
         latency shadow, then ts=T*s, s2=s*s, ts2=ts*ts and the four
         reduces, ordered so dependent pairs are separated by independent
         work (same-engine RAW needs a sem hop of ~100ns).
  Pool : iota the scatter indices, memset the zero strip, then the SWDGE
         PREPARE_ONLY dma_scatter_add - descriptor generation (~1μs Q7
         kernel) runs during the in-DMA dead time - and finally trigger_dma
         once results + zero-DMA land. The triggered transfer replaces the
         out-DMA's ~1.9μs HWDGE front-end with a ~37ns doorbell.

The entry all-engine barrier emitted by Bass.__init__ only orders the
const-AP memsets (Pool) against engine bodies; none of this program's
engines touch the const APs before Pool's memsets retire, so it is stripped
post-build (_strip_entry_barrier), letting the in-DMA issue at t~25 instead
of t~666. The Block exit barrier is kept: the program must not end before
the scatter-DMA's completion semaphore fires (the host reads o right after).
"""

import numpy as np

try:
    import concourse.bass as bass
except ImportError:  # concourse ships in the container, not on sys.path
    import sys

    sys.path.insert(0, "/opt/trn_rl_repo")
    import concourse.bass as bass

from concourse import bacc, mybir
from concourse import bass_utils

N = 16384
NCORES = 8
SHARD = N // NCORES  # 2048 elements per core
P = 128  # SBUF partitions
F = SHARD // P  # 16 free elements per partition
OC = 64  # output dram row stride (256B, scatter-add stride granularity)

f32 = mybir.dt.float32
i16 = mybir.dt.int16

_CACHE: dict = {}


def _strip_entry_barrier(nc):
    """Remove the Drain+EventSemaphore all-engine barrier that Bass.__init__
    appends to the entry block after the const-AP memsets. The memsets stay
    (Pool-only, and this program never reads the const APs)."""
    entry = nc.main_func.blocks[0]
    keep = [
        inst
        for inst in entry.instructions
        if not isinstance(inst, (mybir.InstDrain, mybir.InstEventSemaphore))
    ]
    removed = len(entry.instructions) - len(keep)
    assert removed == 11, f"expected 11 entry-barrier instructions, got {removed}"
    entry.instructions[:] = keep


def _build():
    nc = bacc.Bacc(
        "TRN2",
        target_bir_lowering=False,
        debug=False,
        enable_asserts=False,
        num_devices=NCORES,
    )
    x_dram = nc.dram_tensor("x", [P, 2 * F], f32, kind="ExternalInput").ap()
    o_dram = nc.dram_tensor("o", [P, OC], f32, kind="ExternalOutput").ap()

    Sig = mybir.ActivationFunctionType.Sigmoid
    X = mybir.AxisListType.X

    with (
        nc.sbuf_tensor([P, 2 * F], f32) as x,
        nc.sbuf_tensor([P, F], f32) as s,
        nc.sbuf_tensor([P, F], f32) as s2,
        nc.sbuf_tensor([P, F], f32) as ts,
        nc.sbuf_tensor([P, F], f32) as ts2,
        nc.sbuf_tensor([P, 1, 8], f32) as r,  # g1|g2|c|m1|m2|junk[3]
        nc.sbuf_tensor([P, 5], f32) as zeros,
        nc.sbuf_tensor([16, 8], i16) as idxs,
        nc.semaphore() as dsem,  # in-DMA complete
        nc.semaphore() as msem,  # zeros strip memset done
        nc.semaphore() as ssem,  # sigmoid retired
        nc.semaphore() as wsem,  # DVE intermediate (ts, s2, ts2) retired
        nc.semaphore() as vsem,  # DVE result column retired (5 total)
        nc.semaphore() as isem,  # iota retired
        nc.semaphore() as psem,  # scatter prep descriptors generated
        nc.semaphore() as zsem,  # zero-DMA complete
        nc.semaphore() as osem,  # scatter-DMA complete
        nc.Block() as block,
    ):
        L = x[:, 0:F]
        T = x[:, F : 2 * F]

        @block.sync
        def _(sync):
            sync.dma_start(x[:], x_dram).then_inc(dsem, 16)
            sync.wait_ge(msem, 1)
            sync.dma_start(o_dram[:, 0:5], zeros[:]).then_inc(zsem, 16)
            sync.wait_ge(osem, 16)  # out writes visible before program end

        @block.scalar
        def _(scalar):
            scalar.wait_ge(dsem, 16)
            nc.scalar.activation(s[:], L, Sig).then_inc(ssem, 1)

        @block.vector
        def _(vector):
            vector.wait_ge(dsem, 16)
            # c: independent of s, fills the sigmoid's ~460ns latency shadow
            nc.vector.reduce_sum(r[:, 0, 2:3], T, axis=X).then_inc(vsem, 1)
            vector.wait_ge(ssem, 1)
            nc.vector.tensor_mul(ts[:], T, s[:]).then_inc(wsem, 1)
            nc.vector.tensor_mul(s2[:], s[:], s[:]).then_inc(wsem, 1)
            vector.wait_ge(wsem, 1)  # ts retired
            nc.vector.tensor_mul(ts2[:], ts[:], ts[:]).then_inc(wsem, 1)
            nc.vector.reduce_sum(r[:, 0, 0:1], s[:], axis=X).then_inc(vsem, 1)
            nc.vector.reduce_sum(r[:, 0, 3:4], ts[:], axis=X).then_inc(vsem, 1)
            vector.wait_ge(wsem, 2)  # s2 retired
            nc.vector.reduce_sum(r[:, 0, 1:2], s2[:], axis=X).then_inc(vsem, 1)
            vector.wait_ge(wsem, 3)  # ts2 retired
            nc.vector.reduce_sum(r[:, 0, 4:5], ts2[:], axis=X).then_inc(vsem, 1)

        @block.gpsimd
        def _(gpsimd):
            # idxs[p, j] = p + 16*j: identity scatter, wrapped in 16
            # partitions as the q7 desc-gen kernel expects
            nc.gpsimd.iota(
                idxs[:], pattern=[[16, 8]], base=0, channel_multiplier=1
            ).then_inc(isem, 1)
            nc.gpsimd.memset(zeros[:], 0.0).then_inc(msem, 1)
            gpsimd.wait_ge(isem, 1)
            nc.gpsimd.dma_scatter_add(
                o_dram[:, 0:8],
                r[:],
                idxs[:],
                P,  # num_idxs
                P,  # num_idxs_reg
                8,  # elem_size: 8 f32 payload per partition row
                elem_step=OC,  # dram row stride 64 f32 = 256B
                prepare_only=True,
                sem=osem,
            ).then_inc(psem, 1)
            gpsimd.wait_ge(psem, 1)  # descriptors written to the ring
            gpsimd.wait_ge(zsem, 16)  # o[:,0:5] zeroed (scatter ADDs)
            gpsimd.wait_ge(vsem, 5)  # all five result columns retired
            nc.gpsimd.trigger_dma(count=1)

    _strip_entry_barrier(nc)
    nc.compile()
    return nc


def _get_nc():
    if "nc" not in _CACHE:
        _CACHE["nc"] = _build()
    return _CACHE["nc"]


def make_in_maps(logits: np.ndarray, targets: np.ndarray) -> list[dict]:
    logits = np.ascontiguousarray(logits, dtype=np.float32)
    t32 = np.asarray(targets).astype(np.float32)  # values are 0/1; lossless
    in_maps = []
    for k in range(NCORES):
        sl = slice(k * SHARD, (k + 1) * SHARD)
        xk = np.empty((P, 2 * F), np.float32)
        xk[:, 0:F] = logits[sl].reshape(P, F)
        xk[:, F : 2 * F] = t32[sl].reshape(P, F)
        in_maps.append({"x": xk})
    return in_maps


def combine(outs: np.ndarray) -> np.ndarray:
    """All-reduce the [NCORES, P, 5] partials and apply the closed form."""
    tot = outs.astype(np.float64).sum(axis=(0, 1))
    g1, g2, c, m1, m2 = tot
    n_pos = c
    n_neg = float(N) - c
    sp1 = c - m1
    sp2 = c - 2.0 * m1 + m2
    sn1 = g1 - m1
    sn2 = g2 - m2
    loss = (n_neg * sp2 + 2.0 * sp1 * sn1 + n_pos * sn2) / (n_pos * n_neg)
    return np.array(loss, dtype=np.float32)


def kernel(logits: np.ndarray, targets: np.ndarray, **run_kwargs):
    nc = _get_nc()
    res = bass_utils.run_bass_kernel_spmd(
        nc, make_in_maps(logits, targets), core_ids=list(range(NCORES)), **run_kwargs
    )
    outs = np.stack([r["o"][:, 0:5] for r in res.results])  # [8, 128, 5]
    out = combine(outs)
    _CACHE["last_results"] = res
    return out


# revision 9
# speedup vs baseline: 1.4091x; 1.0038x over previous
"""Trainium2 Bass kernel for the MiniBatchAUC pairwise surrogate loss.

Math: with s = sigmoid(logits), pos/neg the 0/1 target masks,
    loss_sum = sum_{i in P, j in N} (1 - s_i + s_j)^2
factorizes exactly (expand the square; the double sum separates):
    loss_sum = n_neg * Sp2 + 2 * Sp1 * Sn1 + n_pos * Sn2
      Sp1 = sum_P (1-s),  Sp2 = sum_P (1-s)^2,
      Sn1 = sum_N s,      Sn2 = sum_N s^2,
and with c = sum T, m1 = sum T*s, m2 = sum T*s^2, g1 = sum s, g2 = sum s^2:
      Sp1 = c - m1, Sp2 = c - 2*m1 + m2, Sn1 = g1 - m1, Sn2 = g2 - m2.
So the O(N^2) pairwise matrix is never materialized: each core reduces its
2048-element shard to 5 per-partition partial sums; the host all-reduces
the per-core partials and applies the closed form.

Per-core device program (SPMD, identical on all 8 cores), raw bacc with
manual semaphores. The schedule is built around the three fixed DMA
latencies the TRN2 cost model charges (HWDGE descriptor generation ~625ns,
DGE->DMA-engine delay ~650ns, DMA-completion->semaphore propagation ~900ns):

  SP   : in-DMA [128,32] emitted in the ENTRY block so it issues at t~0
         (no body-branch in front); then a zero-DMA clearing o[:,0:5]
         (output DRAM is NOT zero-initialized - verified empirically - and
         the output path is a scatter-ADD); finally waits on the
         scatter-DMA completion sem so the program cannot end before the
         output writes are globally visible.
  ACT  : one sigmoid over the 16 logit columns, no accum_out (the
         accumulator read costs an extra 187ns engine slice and delays the
         semaphore every consumer waits on).
  DVE  : c = reduce(T) inside the sigmoid's ~460ns latency shadow, then
         ts = T*s, s2 = s*s, g1 = reduce(s), m1 = reduce(ts),
         g2 = reduce(s2) - ordered so every dependent pair is separated by
         independent work (same-engine RAW costs a ~100ns sem hop).
  Pool : memset of the zero strip, iota of the scatter indices, then the
         SWDGE PREPARE_ONLY dma_scatter_add - the ~1μs Q7 descriptor
         generation runs entirely inside the in-DMA dead time - then the
         depth-3 tail (ts2 = ts*ts, m2 = reduce(ts2)) in parallel with
         DVE's reduces, and finally trigger_dma once all five result
         columns, the zero-DMA, and the prep have retired. The triggered
         transfer replaces a store-DMA's ~1.9μs HWDGE front-end with a
         ~37ns doorbell. (tensor_tensor_reduce would fuse mul+reduce but
         crashes this runtime - re-verified this session.)

The entry all-engine barrier emitted by Bass.__init__ only orders the
const-AP memsets (Pool) against engine bodies; nothing here reads the const
APs, so it is stripped post-build (saves ~640ns of dead startup). The Block
exit barrier (per-engine Drain + EVSEM handshake) is also stripped when
STRIP_EXIT=True: every DMA the program issues is semaphore-quiesced before
SP's final wait, so engines may retire independently.
"""

import numpy as np

try:
    import concourse.bass as bass
except ImportError:  # concourse ships in the container, not on sys.path
    import sys

    sys.path.insert(0, "/opt/trn_rl_repo")
    import concourse.bass as bass

from concourse import bacc, mybir
from concourse import bass_utils

N = 16384
NCORES = 8
SHARD = N // NCORES  # 2048 elements per core
P = 128  # SBUF partitions
F = SHARD // P  # 16 free elements per partition
OC = 64  # output dram row stride (256B, scatter-add stride granularity)

STRIP_EXIT = False

f32 = mybir.dt.float32
i16 = mybir.dt.int16

_CACHE: dict = {}


def _strip_barriers(nc):
    """Remove the Drain+EventSemaphore all-engine barrier that Bass.__init__
    appends to the entry block (the const-AP memsets stay: Pool-only, and
    this program never reads the const APs), and optionally the Block exit
    barrier (all issued DMAs are already sem-quiesced before program end)."""
    entry = nc.main_func.blocks[0]
    keep = [
        inst
        for inst in entry.instructions
        if not isinstance(inst, (mybir.InstDrain, mybir.InstEventSemaphore))
    ]
    removed = len(entry.instructions) - len(keep)
    assert removed == 11, f"expected 11 entry-barrier instructions, got {removed}"
    entry.instructions[:] = keep

    if STRIP_EXIT:
        end = next(b for b in nc.main_func.blocks if b.name.endswith("_end"))
        keep = [
            inst
            for inst in end.instructions
            if not isinstance(inst, (mybir.InstDrain, mybir.InstEventSemaphore))
        ]
        end.instructions[:] = keep


def _build():
    nc = bacc.Bacc(
        "TRN2",
        target_bir_lowering=False,
        debug=False,
        enable_asserts=False,
        num_devices=NCORES,
    )
    x_dram = nc.dram_tensor("x", [P, 2 * F], f32, kind="ExternalInput").ap()
    o_dram = nc.dram_tensor("o", [P, OC], f32, kind="ExternalOutput").ap()

    Sig = mybir.ActivationFunctionType.Sigmoid
    X = mybir.AxisListType.X

    with (
        nc.sbuf_tensor([P, 2 * F], f32) as x,
        nc.sbuf_tensor([P, F], f32) as s,
        nc.sbuf_tensor([P, F], f32) as s2,
        nc.sbuf_tensor([P, F], f32) as ts,
        nc.sbuf_tensor([P, F], f32) as ts2,
        nc.sbuf_tensor([P, 1, 8], f32) as r,  # g1|g2|c|m1|m2|junk[3]
        nc.sbuf_tensor([P, 5], f32) as zeros,
        nc.sbuf_tensor([16, 8], i16) as idxs,
        nc.semaphore() as dsem,  # in-DMA complete
        nc.semaphore() as msem,  # zeros strip memset done
        nc.semaphore() as ssem,  # sigmoid retired
        nc.semaphore() as wsem,  # DVE intermediates (ts, u2) retired
        nc.semaphore() as vsem,  # result columns retired (5 total)
        nc.semaphore() as isem,  # iota retired
        nc.semaphore() as psem,  # scatter prep descriptors generated
        nc.semaphore() as zsem,  # zero-DMA complete
        nc.semaphore() as osem,  # scatter-DMA complete
    ):
        L = x[:, 0:F]
        T = x[:, F : 2 * F]

        with nc.Block() as block:

            @block.sync
            def _(sync):
                sync.dma_start(x[:], x_dram).then_inc(dsem, 16)
                sync.wait_ge(msem, 1)
                sync.dma_start(o_dram[:, 0:5], zeros[:]).then_inc(zsem, 16)
                sync.wait_ge(osem, 16)  # out writes visible before end

            @block.scalar
            def _(scalar):
                scalar.wait_ge(dsem, 16)
                nc.scalar.activation(s[:], L, Sig).then_inc(ssem, 1)

            @block.vector
            def _(vector):
                vector.wait_ge(dsem, 16)
                # c: independent of s, fills the sigmoid latency shadow
                nc.vector.reduce_sum(r[:, 0, 2:3], T, axis=X).then_inc(vsem, 1)
                vector.wait_ge(ssem, 1)
                nc.vector.tensor_mul(ts[:], T, s[:]).then_inc(wsem, 1)
                nc.vector.tensor_mul(s2[:], s[:], s[:]).then_inc(wsem, 1)
                nc.vector.reduce_sum(r[:, 0, 0:1], s[:], axis=X).then_inc(vsem, 1)
                vector.wait_ge(wsem, 1)  # ts retired
                nc.vector.tensor_mul(ts2[:], ts[:], ts[:]).then_inc(wsem, 1)
                nc.vector.reduce_sum(r[:, 0, 3:4], ts[:], axis=X).then_inc(vsem, 1)
                vector.wait_ge(wsem, 2)  # s2 retired
                nc.vector.reduce_sum(r[:, 0, 1:2], s2[:], axis=X).then_inc(vsem, 1)
                vector.wait_ge(wsem, 3)  # ts2 retired
                nc.vector.reduce_sum(r[:, 0, 4:5], ts2[:], axis=X).then_inc(vsem, 1)

            @block.gpsimd
            def _(gpsimd):
                nc.gpsimd.memset(zeros[:], 0.0).then_inc(msem, 1)
                # idxs[p, j] = p + 16*j: identity scatter, wrapped in 16
                # partitions as the q7 desc-gen kernel expects
                nc.gpsimd.iota(
                    idxs[:], pattern=[[16, 8]], base=0, channel_multiplier=1
                ).then_inc(isem, 1)
                gpsimd.wait_ge(isem, 1)
                nc.gpsimd.dma_scatter_add(
                    o_dram[:, 0:8],
                    r[:],
                    idxs[:],
                    P,  # num_idxs
                    P,  # num_idxs_reg
                    8,  # elem_size: 8 f32 payload per partition row
                    elem_step=OC,  # dram row stride 64 f32 = 256B
                    prepare_only=True,
                    sem=osem,
                ).then_inc(psem, 1)
                gpsimd.wait_ge(psem, 1)  # descriptors written to the ring
                gpsimd.wait_ge(zsem, 16)  # o[:,0:5] zeroed (scatter ADDs)
                gpsimd.wait_ge(vsem, 5)  # all five result columns retired
                nc.gpsimd.trigger_dma(count=1)

    _strip_barriers(nc)
    nc.compile()
    return nc


def _get_nc():
    if "nc" not in _CACHE:
        _CACHE["nc"] = _build()
    return _CACHE["nc"]


def make_in_maps(logits: np.ndarray, targets: np.ndarray) -> list[dict]:
    logits = np.ascontiguousarray(logits, dtype=np.float32)
    t32 = np.asarray(targets).astype(np.float32)  # values are 0/1; lossless
    in_maps = []
    for k in range(NCORES):
        sl = slice(k * SHARD, (k + 1) * SHARD)
        xk = np.empty((P, 2 * F), np.float32)
        xk[:, 0:F] = logits[sl].reshape(P, F)
        xk[:, F : 2 * F] = t32[sl].reshape(P, F)
        in_maps.append({"x": xk})
    return in_maps


def combine(outs: np.ndarray) -> np.ndarray:
    """All-reduce the [NCORES, P, 5] partials and apply the closed form."""
    tot = outs.astype(np.float64).sum(axis=(0, 1))
    g1, g2, c, m1, m2 = tot
    n_pos = c
    n_neg = float(N) - c
    sp1 = c - m1
    sp2 = c - 2.0 * m1 + m2
    sn1 = g1 - m1
    sn2 = g2 - m2
    loss = (n_neg * sp2 + 2.0 * sp1 * sn1 + n_pos * sn2) / (n_pos * n_neg)
    return np.array(loss, dtype=np.float32)


def kernel(logits: np.ndarray, targets: np.ndarray, **run_kwargs):
    nc = _get_nc()
    res = bass_utils.run_bass_kernel_spmd(
        nc, make_in_maps(logits, targets), core_ids=list(range(NCORES)), **run_kwargs
    )
    outs = np.stack([r["o"][:, 0:5] for r in res.results])  # [8, 128, 5]
    out = combine(outs)
    _CACHE["last_results"] = res
    return out


# revision 10
# speedup vs baseline: 1.4830x; 1.0524x over previous
"""Trainium2 Bass kernel for the MiniBatchAUC pairwise surrogate loss.

Math: with s = sigmoid(logits), pos/neg the 0/1 target masks,
    loss_sum = sum_{i in P, j in N} (1 - s_i + s_j)^2
factorizes exactly (expand the square; the double sum separates):
    loss_sum = n_neg * Sp2 + 2 * Sp1 * Sn1 + n_pos * Sn2
      Sp1 = sum_P (1-s),  Sp2 = sum_P (1-s)^2,
      Sn1 = sum_N s,      Sn2 = sum_N s^2,
and with c = sum T, m1 = sum T*s, m2 = sum T*s^2, g1 = sum s, g2 = sum s^2:
      Sp1 = c - m1, Sp2 = c - 2*m1 + m2, Sn1 = g1 - m1, Sn2 = g2 - m2.
So the O(N^2) pairwise matrix is never materialized: each core reduces its
2048-element shard to 5 per-partition partial sums; the host all-reduces
the per-core partials and applies the closed form.

Per-core device program (SPMD, identical on all 8 cores), raw bacc with
manual semaphores. The schedule is built around the three fixed DMA
latencies the TRN2 cost model charges (HWDGE descriptor generation ~625ns,
DGE->DMA-engine delay ~650ns, DMA-completion->semaphore propagation ~900ns):

  SP   : in-DMA [128,32] emitted in the ENTRY block so it issues at t~0
         (no body-branch in front); then a zero-DMA clearing o[:,0:5]
         (output DRAM is NOT zero-initialized - verified empirically - and
         the output path is a scatter-ADD); finally waits on the
         scatter-DMA completion sem so the program cannot end before the
         output writes are globally visible.
  ACT  : one sigmoid over the 16 logit columns, no accum_out (the
         accumulator read costs an extra 187ns engine slice and delays the
         semaphore every consumer waits on).
  DVE  : c = reduce(T) inside the sigmoid's ~460ns latency shadow, then
         ts = T*s, s2 = s*s, g1 = reduce(s), m1 = reduce(ts),
         g2 = reduce(s2) - ordered so every dependent pair is separated by
         independent work (same-engine RAW costs a ~100ns sem hop).
  Pool : memset of the zero strip, iota of the scatter indices, then the
         SWDGE PREPARE_ONLY dma_scatter_add - the ~1μs Q7 descriptor
         generation runs entirely inside the in-DMA dead time - then the
         depth-3 tail (ts2 = ts*ts, m2 = reduce(ts2)) in parallel with
         DVE's reduces, and finally trigger_dma once all five result
         columns, the zero-DMA, and the prep have retired. The triggered
         transfer replaces a store-DMA's ~1.9μs HWDGE front-end with a
         ~37ns doorbell. (tensor_tensor_reduce would fuse mul+reduce but
         crashes this runtime - re-verified this session.)

The entry all-engine barrier emitted by Bass.__init__ only orders the
const-AP memsets (Pool) against engine bodies; nothing here reads the const
APs, so it is stripped post-build (saves ~640ns of dead startup). The Block
exit barrier (per-engine Drain + EVSEM handshake) is also stripped when
STRIP_EXIT=True: every DMA the program issues is semaphore-quiesced before
SP's final wait, so engines may retire independently.
"""

import numpy as np

try:
    import concourse.bass as bass
except ImportError:  # concourse ships in the container, not on sys.path
    import sys

    sys.path.insert(0, "/opt/trn_rl_repo")
    import concourse.bass as bass

from concourse import bacc, mybir
from concourse import bass_utils

N = 16384
NCORES = 8
SHARD = N // NCORES  # 2048 elements per core
P = 128  # SBUF partitions
F = SHARD // P  # 16 free elements per partition
OC = 64  # output dram row stride (256B, scatter-add stride granularity)

STRIP_EXIT = True

f32 = mybir.dt.float32
i16 = mybir.dt.int16

_CACHE: dict = {}


def _strip_barriers(nc):
    """Remove the Drain+EventSemaphore all-engine barrier that Bass.__init__
    appends to the entry block (the const-AP memsets stay: Pool-only, and
    this program never reads the const APs), and optionally the Block exit
    barrier (all issued DMAs are already sem-quiesced before program end)."""
    entry = nc.main_func.blocks[0]
    keep = [
        inst
        for inst in entry.instructions
        if not isinstance(inst, (mybir.InstDrain, mybir.InstEventSemaphore))
    ]
    removed = len(entry.instructions) - len(keep)
    assert removed == 11, f"expected 11 entry-barrier instructions, got {removed}"
    entry.instructions[:] = keep

    if STRIP_EXIT:
        end = next(b for b in nc.main_func.blocks if b.name.endswith("_end"))
        keep = [
            inst
            for inst in end.instructions
            if not isinstance(inst, (mybir.InstDrain, mybir.InstEventSemaphore))
        ]
        end.instructions[:] = keep


def _build():
    nc = bacc.Bacc(
        "TRN2",
        target_bir_lowering=False,
        debug=False,
        enable_asserts=False,
        num_devices=NCORES,
    )
    x_dram = nc.dram_tensor("x", [P, 2 * F], f32, kind="ExternalInput").ap()
    o_dram = nc.dram_tensor("o", [P, OC], f32, kind="ExternalOutput").ap()

    Sig = mybir.ActivationFunctionType.Sigmoid
    X = mybir.AxisListType.X

    with (
        nc.sbuf_tensor([P, 2 * F], f32) as x,
        nc.sbuf_tensor([P, F], f32) as s,
        nc.sbuf_tensor([P, F], f32) as s2,
        nc.sbuf_tensor([P, F], f32) as ts,
        nc.sbuf_tensor([P, F], f32) as ts2,
        nc.sbuf_tensor([P, 1, 8], f32) as r,  # g1|g2|c|m1|m2|junk[3]
        nc.sbuf_tensor([P, 5], f32) as zeros,
        nc.sbuf_tensor([16, 8], i16) as idxs,
        nc.semaphore() as dsem,  # in-DMA complete
        nc.semaphore() as msem,  # zeros strip memset done
        nc.semaphore() as ssem,  # sigmoid retired
        nc.semaphore() as wsem,  # DVE intermediates (ts, u2) retired
        nc.semaphore() as vsem,  # result columns retired (5 total)
        nc.semaphore() as isem,  # iota retired
        nc.semaphore() as psem,  # scatter prep descriptors generated
        nc.semaphore() as zsem,  # zero-DMA complete
        nc.semaphore() as osem,  # scatter-DMA complete
    ):
        L = x[:, 0:F]
        T = x[:, F : 2 * F]

        with nc.Block() as block:

            @block.sync
            def _(sync):
                sync.dma_start(x[:], x_dram).then_inc(dsem, 16)
                sync.wait_ge(msem, 1)
                sync.dma_start(o_dram[:, 0:5], zeros[:]).then_inc(zsem, 16)
                sync.wait_ge(osem, 16)  # out writes visible before end

            @block.scalar
            def _(scalar):
                scalar.wait_ge(dsem, 16)
                nc.scalar.activation(s[:], L, Sig).then_inc(ssem, 1)

            @block.vector
            def _(vector):
                vector.wait_ge(dsem, 16)
                # c: independent of s, fills the sigmoid latency shadow
                nc.vector.reduce_sum(r[:, 0, 2:3], T, axis=X).then_inc(vsem, 1)
                vector.wait_ge(ssem, 1)
                nc.vector.tensor_mul(ts[:], T, s[:]).then_inc(wsem, 1)
                nc.vector.tensor_mul(s2[:], s[:], s[:]).then_inc(wsem, 1)
                nc.vector.reduce_sum(r[:, 0, 0:1], s[:], axis=X).then_inc(vsem, 1)
                vector.wait_ge(wsem, 1)  # ts retired
                nc.vector.tensor_mul(ts2[:], ts[:], ts[:]).then_inc(wsem, 1)
                nc.vector.reduce_sum(r[:, 0, 3:4], ts[:], axis=X).then_inc(vsem, 1)
                vector.wait_ge(wsem, 2)  # s2 retired
                nc.vector.reduce_sum(r[:, 0, 1:2], s2[:], axis=X).then_inc(vsem, 1)
                vector.wait_ge(wsem, 3)  # ts2 retired
                nc.vector.reduce_sum(r[:, 0, 4:5], ts2[:], axis=X).then_inc(vsem, 1)

            @block.gpsimd
            def _(gpsimd):
                nc.gpsimd.memset(zeros[:], 0.0).then_inc(msem, 1)
                # idxs[p, j] = p + 16*j: identity scatter, wrapped in 16
                # partitions as the q7 desc-gen kernel expects
                nc.gpsimd.iota(
                    idxs[:], pattern=[[16, 8]], base=0, channel_multiplier=1
                ).then_inc(isem, 1)
                gpsimd.wait_ge(isem, 1)
                nc.gpsimd.dma_scatter_add(
                    o_dram[:, 0:8],
                    r[:],
                    idxs[:],
                    P,  # num_idxs
                    P,  # num_idxs_reg
                    8,  # elem_size: 8 f32 payload per partition row
                    elem_step=OC,  # dram row stride 64 f32 = 256B
                    prepare_only=True,
                    sem=osem,
                ).then_inc(psem, 1)
                gpsimd.wait_ge(psem, 1)  # descriptors written to the ring
                gpsimd.wait_ge(zsem, 16)  # o[:,0:5] zeroed (scatter ADDs)
                gpsimd.wait_ge(vsem, 5)  # all five result columns retired
                nc.gpsimd.trigger_dma(count=1)

    _strip_barriers(nc)
    nc.compile()
    return nc


def _get_nc():
    if "nc" not in _CACHE:
        _CACHE["nc"] = _build()
    return _CACHE["nc"]


def make_in_maps(logits: np.ndarray, targets: np.ndarray) -> list[dict]:
    logits = np.ascontiguousarray(logits, dtype=np.float32)
    t32 = np.asarray(targets).astype(np.float32)  # values are 0/1; lossless
    in_maps = []
    for k in range(NCORES):
        sl = slice(k * SHARD, (k + 1) * SHARD)
        xk = np.empty((P, 2 * F), np.float32)
        xk[:, 0:F] = logits[sl].reshape(P, F)
        xk[:, F : 2 * F] = t32[sl].reshape(P, F)
        in_maps.append({"x": xk})
    return in_maps


def combine(outs: np.ndarray) -> np.ndarray:
    """All-reduce the [NCORES, P, 5] partials and apply the closed form."""
    tot = outs.astype(np.float64).sum(axis=(0, 1))
    g1, g2, c, m1, m2 = tot
    n_pos = c
    n_neg = float(N) - c
    sp1 = c - m1
    sp2 = c - 2.0 * m1 + m2
    sn1 = g1 - m1
    sn2 = g2 - m2
    loss = (n_neg * sp2 + 2.0 * sp1 * sn1 + n_pos * sn2) / (n_pos * n_neg)
    return np.array(loss, dtype=np.float32)


def kernel(logits: np.ndarray, targets: np.ndarray, **run_kwargs):
    nc = _get_nc()
    res = bass_utils.run_bass_kernel_spmd(
        nc, make_in_maps(logits, targets), core_ids=list(range(NCORES)), **run_kwargs
    )
    outs = np.stack([r["o"][:, 0:5] for r in res.results])  # [8, 128, 5]
    out = combine(outs)
    _CACHE["last_results"] = res
    return out


# revision 11
# speedup vs baseline: 1.4999x; 1.0114x over previous
"""Trainium2 Bass kernel for the MiniBatchAUC pairwise surrogate loss.

Math: with s = sigmoid(logits), pos/neg the 0/1 target masks,
    loss_sum = sum_{i in P, j in N} (1 - s_i + s_j)^2
factorizes exactly (expand the square; the double sum separates):
    loss_sum = n_neg * Sp2 + 2 * Sp1 * Sn1 + n_pos * Sn2
      Sp1 = sum_P (1-s),  Sp2 = sum_P (1-s)^2,
      Sn1 = sum_N s,      Sn2 = sum_N s^2,
and with c = sum T, m1 = sum T*s, m2 = sum T*s^2, g1 = sum s, g2 = sum s^2:
      Sp1 = c - m1, Sp2 = c - 2*m1 + m2, Sn1 = g1 - m1, Sn2 = g2 - m2.
So the O(N^2) pairwise matrix is never materialized: each core reduces its
2048-element shard to 5 per-partition partial sums; the host all-reduces
the per-core partials and applies the closed form.

Per-core device program (SPMD, identical on all 8 cores), raw bacc with
manual semaphores. The schedule is built around the three fixed DMA
latencies the TRN2 cost model charges (HWDGE descriptor generation ~625ns,
DGE->DMA-engine delay ~650ns, DMA-completion->semaphore propagation ~900ns):

  SP   : in-DMA [128,32] emitted in the ENTRY block so it issues at t~0
         (no body-branch in front); then a zero-DMA clearing o[:,0:5]
         (output DRAM is NOT zero-initialized - verified empirically - and
         the output path is a scatter-ADD); finally waits on the
         scatter-DMA completion sem so the program cannot end before the
         output writes are globally visible.
  ACT  : one sigmoid over the 16 logit columns, no accum_out (the
         accumulator read costs an extra 187ns engine slice and delays the
         semaphore every consumer waits on).
  DVE  : c = reduce(T) inside the sigmoid's ~460ns latency shadow, then
         ts = T*s, s2 = s*s, g1 = reduce(s), m1 = reduce(ts),
         g2 = reduce(s2) - ordered so every dependent pair is separated by
         independent work (same-engine RAW costs a ~100ns sem hop).
  Pool : memset of the zero strip, iota of the scatter indices, then the
         SWDGE PREPARE_ONLY dma_scatter_add - the ~1μs Q7 descriptor
         generation runs entirely inside the in-DMA dead time - then the
         depth-3 tail (ts2 = ts*ts, m2 = reduce(ts2)) in parallel with
         DVE's reduces, and finally trigger_dma once all five result
         columns, the zero-DMA, and the prep have retired. The triggered
         transfer replaces a store-DMA's ~1.9μs HWDGE front-end with a
         ~37ns doorbell. (tensor_tensor_reduce would fuse mul+reduce but
         crashes this runtime - re-verified this session.)

The entry all-engine barrier emitted by Bass.__init__ only orders the
const-AP memsets (Pool) against engine bodies; nothing here reads the const
APs, so it is stripped post-build (saves ~640ns of dead startup). The Block
exit barrier (per-engine Drain + EVSEM handshake) is also stripped when
STRIP_EXIT=True: every DMA the program issues is semaphore-quiesced before
SP's final wait, so engines may retire independently.
"""

import numpy as np

try:
    import concourse.bass as bass
except ImportError:  # concourse ships in the container, not on sys.path
    import sys

    sys.path.insert(0, "/opt/trn_rl_repo")
    import concourse.bass as bass

from concourse import bacc, mybir
from concourse import bass_utils

N = 16384
NCORES = 8
SHARD = N // NCORES  # 2048 elements per core
P = 128  # SBUF partitions
F = SHARD // P  # 16 free elements per partition
OC = 64  # output dram row stride (256B, scatter-add stride granularity)

STRIP_EXIT = True

f32 = mybir.dt.float32
i16 = mybir.dt.int16

_CACHE: dict = {}


def _strip_barriers(nc):
    """Remove the Drain+EventSemaphore all-engine barrier that Bass.__init__
    appends to the entry block (the const-AP memsets stay: Pool-only, and
    this program never reads the const APs), and optionally the Block exit
    barrier (all issued DMAs are already sem-quiesced before program end)."""
    entry = nc.main_func.blocks[0]
    keep = [
        inst
        for inst in entry.instructions
        if not isinstance(inst, (mybir.InstDrain, mybir.InstEventSemaphore))
    ]
    removed = len(entry.instructions) - len(keep)
    assert removed == 11, f"expected 11 entry-barrier instructions, got {removed}"
    entry.instructions[:] = keep

    if STRIP_EXIT:
        end = next(b for b in nc.main_func.blocks if b.name.endswith("_end"))
        keep = [
            inst
            for inst in end.instructions
            if not isinstance(inst, (mybir.InstDrain, mybir.InstEventSemaphore))
        ]
        end.instructions[:] = keep


def _build():
    nc = bacc.Bacc(
        "TRN2",
        target_bir_lowering=False,
        debug=False,
        enable_asserts=False,
        num_devices=NCORES,
    )
    x_dram = nc.dram_tensor("x", [P, 2 * F], f32, kind="ExternalInput").ap()
    o_dram = nc.dram_tensor("o", [P, OC], f32, kind="ExternalOutput").ap()

    Sig = mybir.ActivationFunctionType.Sigmoid
    X = mybir.AxisListType.X

    with (
        nc.sbuf_tensor([P, 2 * F], f32) as x,
        nc.sbuf_tensor([P, F], f32) as s,
        nc.sbuf_tensor([P, F], f32) as s2,
        nc.sbuf_tensor([P, F], f32) as ts,
        nc.sbuf_tensor([P, F], f32) as ts2,
        nc.sbuf_tensor([P, 1, 8], f32) as r,  # g1|g2|c|m1|m2|junk[3]
        nc.sbuf_tensor([P, 5], f32) as zeros,
        nc.sbuf_tensor([16, 8], i16) as idxs,
        nc.semaphore() as dsem,  # in-DMA complete
        nc.semaphore() as msem,  # zeros strip memset done
        nc.semaphore() as ssem,  # sigmoid retired
        nc.semaphore() as wsem,  # DVE intermediates (ts, u2) retired
        nc.semaphore() as vsem,  # result columns retired (5 total)
        nc.semaphore() as isem,  # iota retired
        nc.semaphore() as psem,  # scatter prep descriptors generated
        nc.semaphore() as zsem,  # zero-DMA complete
        nc.semaphore() as osem,  # scatter-DMA complete
    ):
        L = x[:, 0:F]
        T = x[:, F : 2 * F]

        # Entry-block emission: issues before the per-engine body branches,
        # putting the in-DMA's HWDGE front-end at t~25 instead of t~75.
        nc.sync.dma_start(x[:], x_dram).then_inc(dsem, 16)

        with nc.Block() as block:

            @block.sync
            def _(sync):
                sync.wait_ge(msem, 1)
                sync.dma_start(o_dram[:, 0:5], zeros[:]).then_inc(zsem, 16)
                sync.wait_ge(osem, 16)  # out writes visible before end

            @block.scalar
            def _(scalar):
                scalar.wait_ge(dsem, 16)
                nc.scalar.activation(s[:], L, Sig).then_inc(ssem, 1)

            @block.vector
            def _(vector):
                vector.wait_ge(dsem, 16)
                # c: independent of s, fills the sigmoid latency shadow
                nc.vector.reduce_sum(r[:, 0, 2:3], T, axis=X).then_inc(vsem, 1)
                vector.wait_ge(ssem, 1)
                nc.vector.tensor_mul(ts[:], T, s[:]).then_inc(wsem, 1)
                nc.vector.tensor_mul(s2[:], s[:], s[:]).then_inc(wsem, 1)
                nc.vector.reduce_sum(r[:, 0, 0:1], s[:], axis=X).then_inc(vsem, 1)
                vector.wait_ge(wsem, 1)  # ts retired
                nc.vector.tensor_mul(ts2[:], ts[:], ts[:]).then_inc(wsem, 1)
                nc.vector.reduce_sum(r[:, 0, 3:4], ts[:], axis=X).then_inc(vsem, 1)
                vector.wait_ge(wsem, 2)  # s2 retired
                nc.vector.reduce_sum(r[:, 0, 1:2], s2[:], axis=X).then_inc(vsem, 1)
                vector.wait_ge(wsem, 3)  # ts2 retired
                nc.vector.reduce_sum(r[:, 0, 4:5], ts2[:], axis=X).then_inc(vsem, 1)

            @block.gpsimd
            def _(gpsimd):
                nc.gpsimd.memset(zeros[:], 0.0).then_inc(msem, 1)
                # idxs[p, j] = p + 16*j: identity scatter, wrapped in 16
                # partitions as the q7 desc-gen kernel expects
                nc.gpsimd.iota(
                    idxs[:], pattern=[[16, 8]], base=0, channel_multiplier=1
                ).then_inc(isem, 1)
                gpsimd.wait_ge(isem, 1)
                nc.gpsimd.dma_scatter_add(
                    o_dram[:, 0:8],
                    r[:],
                    idxs[:],
                    P,  # num_idxs
                    P,  # num_idxs_reg
                    8,  # elem_size: 8 f32 payload per partition row
                    elem_step=OC,  # dram row stride 64 f32 = 256B
                    prepare_only=True,
                    sem=osem,
                ).then_inc(psem, 1)
                gpsimd.wait_ge(psem, 1)  # descriptors written to the ring
                gpsimd.wait_ge(zsem, 16)  # o[:,0:5] zeroed (scatter ADDs)
                gpsimd.wait_ge(vsem, 5)  # all five result columns retired
                nc.gpsimd.trigger_dma(count=1)

    _strip_barriers(nc)
    nc.compile()
    return nc


def _get_nc():
    if "nc" not in _CACHE:
        _CACHE["nc"] = _build()
    return _CACHE["nc"]


def make_in_maps(logits: np.ndarray, targets: np.ndarray) -> list[dict]:
    logits = np.ascontiguousarray(logits, dtype=np.float32)
    t32 = np.asarray(targets).astype(np.float32)  # values are 0/1; lossless
    in_maps = []
    for k in range(NCORES):
        sl = slice(k * SHARD, (k + 1) * SHARD)
        xk = np.empty((P, 2 * F), np.float32)
        xk[:, 0:F] = logits[sl].reshape(P, F)
        xk[:, F : 2 * F] = t32[sl].reshape(P, F)
        in_maps.append({"x": xk})
    return in_maps


def combine(outs: np.ndarray) -> np.ndarray:
    """All-reduce the [NCORES, P, 5] partials and apply the closed form."""
    tot = outs.astype(np.float64).sum(axis=(0, 1))
    g1, g2, c, m1, m2 = tot
    n_pos = c
    n_neg = float(N) - c
    sp1 = c - m1
    sp2 = c - 2.0 * m1 + m2
    sn1 = g1 - m1
    sn2 = g2 - m2
    loss = (n_neg * sp2 + 2.0 * sp1 * sn1 + n_pos * sn2) / (n_pos * n_neg)
    return np.array(loss, dtype=np.float32)


def kernel(logits: np.ndarray, targets: np.ndarray, **run_kwargs):
    nc = _get_nc()
    res = bass_utils.run_bass_kernel_spmd(
        nc, make_in_maps(logits, targets), core_ids=list(range(NCORES)), **run_kwargs
    )
    outs = np.stack([r["o"][:, 0:5] for r in res.results])  # [8, 128, 5]
    out = combine(outs)
    _CACHE["last_results"] = res
    return out


# revision 17
# speedup vs baseline: 1.5203x; 1.0136x over previous
"""Trainium2 Bass kernel for the MiniBatchAUC pairwise surrogate loss.

Math: with s = sigmoid(logits), pos/neg the 0/1 target masks,
    loss_sum = sum_{i in P, j in N} (1 - s_i + s_j)^2
factorizes exactly (expand the square; the double sum separates):
    loss_sum = n_neg * Sp2 + 2 * Sp1 * Sn1 + n_pos * Sn2
      Sp1 = sum_P (1-s),  Sp2 = sum_P (1-s)^2,
      Sn1 = sum_N s,      Sn2 = sum_N s^2,
and with c = sum T, m1 = sum T*s, m2 = sum T*s^2, g1 = sum s, g2 = sum s^2:
      Sp1 = c - m1, Sp2 = c - 2*m1 + m2, Sn1 = g1 - m1, Sn2 = g2 - m2.
So the O(N^2) pairwise matrix is never materialized: each core reduces its
2048-element shard to 5 per-partition partial sums; the host all-reduces
the per-core partials and applies the closed form.

Per-core device program (SPMD, identical on all 8 cores), raw bacc with
manual semaphores. The schedule is built around the three fixed DMA
latencies the TRN2 cost model charges (HWDGE descriptor generation ~625ns,
DGE->DMA-engine delay ~650ns, DMA-completion->semaphore propagation ~900ns):

  SP   : in-DMA [128,32] emitted in the ENTRY block so it issues at t~0
         (no body-branch in front); then a zero-DMA clearing o[:,0:5]
         (output DRAM is NOT zero-initialized - verified empirically - and
         the output path is a scatter-ADD); finally waits on the
         scatter-DMA completion sem so the program cannot end before the
         output writes are globally visible.
  ACT  : one sigmoid over the 16 logit columns, no accum_out (the
         accumulator read costs an extra 187ns engine slice and delays the
         semaphore every consumer waits on).
  DVE  : c = reduce(T) inside the sigmoid's ~460ns latency shadow, then
         ts = T*s, s2 = s*s, g1 = reduce(s), m1 = reduce(ts),
         g2 = reduce(s2) - ordered so every dependent pair is separated by
         independent work (same-engine RAW costs a ~100ns sem hop).
  Pool : memset of the zero strip, iota of the scatter indices, then the
         SWDGE PREPARE_ONLY dma_scatter_add - the ~1μs Q7 descriptor
         generation runs entirely inside the in-DMA dead time - then the
         depth-3 tail (ts2 = ts*ts, m2 = reduce(ts2)) in parallel with
         DVE's reduces, and finally trigger_dma once all five result
         columns, the zero-DMA, and the prep have retired. The triggered
         transfer replaces a store-DMA's ~1.9μs HWDGE front-end with a
         ~37ns doorbell. (tensor_tensor_reduce would fuse mul+reduce but
         crashes this runtime - re-verified this session.)

The entry all-engine barrier emitted by Bass.__init__ only orders the
const-AP memsets (Pool) against engine bodies; nothing here reads the const
APs, so it is stripped post-build (saves ~640ns of dead startup). The Block
exit barrier (per-engine Drain + EVSEM handshake) is also stripped when
STRIP_EXIT=True: every DMA the program issues is semaphore-quiesced before
SP's final wait, so engines may retire independently.
"""

import numpy as np

try:
    import concourse.bass as bass
except ImportError:  # concourse ships in the container, not on sys.path
    import sys

    sys.path.insert(0, "/opt/trn_rl_repo")
    import concourse.bass as bass

from concourse import bacc, mybir
from concourse import bass_utils

N = 16384
NCORES = 8
SHARD = N // NCORES  # 2048 elements per core
P = 128  # SBUF partitions
F = SHARD // P  # 16 free elements per partition
OC = 64  # output dram row stride (256B, scatter-add stride granularity)

STRIP_EXIT = True

f32 = mybir.dt.float32
i16 = mybir.dt.int16

_CACHE: dict = {}


def _strip_barriers(nc):
    """Remove the Drain+EventSemaphore all-engine barrier that Bass.__init__
    appends to the entry block (the const-AP memsets stay: Pool-only, and
    this program never reads the const APs), and optionally the Block exit
    barrier (all issued DMAs are already sem-quiesced before program end)."""
    entry = nc.main_func.blocks[0]
    keep, removed = [], 0
    for inst in entry.instructions:
        # Only the FIRST 11 Drain/EVSEM instructions are the __init__
        # barrier; the program's own mid-entry barrier (emitted after the
        # sem_clear) comes later in the list and must stay.
        if removed < 11 and isinstance(
            inst, (mybir.InstDrain, mybir.InstEventSemaphore)
        ):
            removed += 1
            continue
        keep.append(inst)
    assert removed == 11, f"expected 11 entry-barrier instructions, got {removed}"
    entry.instructions[:] = keep

    if STRIP_EXIT:
        end = next(b for b in nc.main_func.blocks if b.name.endswith("_end"))
        keep = [
            inst
            for inst in end.instructions
            if not isinstance(inst, (mybir.InstDrain, mybir.InstEventSemaphore))
        ]
        end.instructions[:] = keep


def _build():
    nc = bacc.Bacc(
        "TRN2",
        target_bir_lowering=False,
        debug=False,
        enable_asserts=False,
        num_devices=NCORES,
    )
    x_dram = nc.dram_tensor("x", [P, 2 * F], f32, kind="ExternalInput").ap()
    o_dram = nc.dram_tensor("o", [P, OC], f32, kind="ExternalOutput").ap()

    Sig = mybir.ActivationFunctionType.Sigmoid
    X = mybir.AxisListType.X

    with (
        nc.sbuf_tensor([P, 2 * F], f32) as x,
        nc.sbuf_tensor([P, F], f32) as s,
        nc.sbuf_tensor([P, F], f32) as sjunk,
        nc.sbuf_tensor([P, F], f32) as s2,
        nc.sbuf_tensor([P, F], f32) as ts,
        nc.sbuf_tensor([P, F], f32) as ts2,
        nc.sbuf_tensor([P, 1, 8], f32) as r,  # g1|g2|c|m1|m2|junk[3]
        nc.sbuf_tensor([P, 5], f32) as zeros,
        nc.sbuf_tensor([16, 8], i16) as idxs,
        nc.semaphore() as dsem,  # in-DMA complete
        nc.semaphore() as ssem,  # sigmoid retired
        nc.semaphore() as wsem,  # DVE intermediates (ts, u2) retired
        nc.semaphore() as vsem,  # result columns retired (5 total)
        nc.semaphore() as isem,  # iota retired
        nc.semaphore() as psem,  # scatter prep descriptors generated
        nc.semaphore() as zsem,  # zero-DMA complete
        nc.semaphore() as osem,  # scatter-DMA complete
    ):
        L = x[:, 0:F]
        T = x[:, F : 2 * F]

        # --- Entry block: issue the sem-free in-DMA immediately, then reset
        # all kernel semaphores (device sem state persists across NEFF
        # executions - stale values would let every wait_ge fall through and
        # the program free-run on stale data; observed as intermittent
        # corruption). The clear + barrier hide entirely inside the in-DMA's
        # ~2.3μs completion latency: the DMA only ADDS to dsem at ~t+2291,
        # long after the clear retires (~t+650).
        nc.sync.dma_start(x[:], x_dram).then_inc(dsem, 16)
        sems = [dsem, ssem, wsem, vsem, isem, psem, zsem, osem]
        nums = sorted(h.num for h in sems)
        assert nums == list(range(nums[0], nums[0] + len(nums))), nums
        nc.gpsimd.sem_clear(range(nums[0], nums[-1] + 1))
        # zeros strip for the zero-DMA: retired ~1.2μs before the zero-DMA's
        # transfer stage reads it (no semaphore needed - the zero-DMA's
        # fixed ~1.3μs HWDGE front-end is the ordering margin)
        nc.gpsimd.memset(zeros[:], 0.0)
        # Barrier: no engine may process a body wait until the clear is
        # done. SP arrives after the in-DMA issue; Pool arrives last (~800)
        # after its clear+memset; ACT's table load starts right after
        # release, still finishing before the in-DMA lands.
        nc.all_engine_barrier()

        with nc.Block() as block:

            @block.sync
            def _(sync):
                sync.dma_start(o_dram[:, 0:5], zeros[:]).then_inc(zsem, 16)
                sync.wait_ge(osem, 16)  # out writes visible before end

            @block.scalar
            def _(scalar):
                scalar.wait_ge(dsem, 16)
                nc.scalar.activation(s[:], L, Sig).then_inc(ssem, 1)
                # g1 on the otherwise-idle ACT: an independent second sigmoid
                # (reads L, not s - no self-chain) whose fused accumulator
                # yields the per-partition sum; lands well before DVE's tail.
                nc.scalar.activation(
                    sjunk[:], L, Sig, accum_out=r[:, 0, 0:1]
                ).then_inc(vsem, 1)

            @block.vector
            def _(vector):
                vector.wait_ge(dsem, 16)
                # c: independent of s, fills the sigmoid latency shadow
                nc.vector.reduce_sum(r[:, 0, 2:3], T, axis=X).then_inc(vsem, 1)
                vector.wait_ge(ssem, 1)
                nc.vector.tensor_mul(ts[:], T, s[:]).then_inc(wsem, 1)
                nc.vector.tensor_mul(s2[:], s[:], s[:]).then_inc(wsem, 1)
                vector.wait_ge(wsem, 1)  # ts retired
                nc.vector.tensor_mul(ts2[:], ts[:], ts[:]).then_inc(wsem, 1)
                nc.vector.reduce_sum(r[:, 0, 3:4], ts[:], axis=X).then_inc(vsem, 1)
                vector.wait_ge(wsem, 2)  # s2 retired
                nc.vector.reduce_sum(r[:, 0, 1:2], s2[:], axis=X).then_inc(vsem, 1)
                vector.wait_ge(wsem, 3)  # ts2 retired
                nc.vector.reduce_sum(r[:, 0, 4:5], ts2[:], axis=X).then_inc(vsem, 1)

            @block.gpsimd
            def _(gpsimd):
                # idxs[p, j] = p + 16*j: identity scatter, wrapped in 16
                # partitions as the q7 desc-gen kernel expects
                nc.gpsimd.iota(
                    idxs[:], pattern=[[16, 8]], base=0, channel_multiplier=1
                ).then_inc(isem, 1)
                gpsimd.wait_ge(isem, 1)
                nc.gpsimd.dma_scatter_add(
                    o_dram[:, 0:8],
                    r[:],
                    idxs[:],
                    P,  # num_idxs
                    P,  # num_idxs_reg
                    8,  # elem_size: 8 f32 payload per partition row
                    elem_step=OC,  # dram row stride 64 f32 = 256B
                    prepare_only=True,
                    sem=osem,
                ).then_inc(psem, 1)
                gpsimd.wait_ge(psem, 1)  # descriptors written to the ring
                gpsimd.wait_ge(zsem, 16)  # o[:,0:5] zeroed (scatter ADDs)
                gpsimd.wait_ge(vsem, 5)  # all five result columns retired
                nc.gpsimd.trigger_dma(count=1)

    _strip_barriers(nc)
    nc.compile()
    return nc


def _get_nc():
    if "nc" not in _CACHE:
        _CACHE["nc"] = _build()
    return _CACHE["nc"]


def make_in_maps(logits: np.ndarray, targets: np.ndarray) -> list[dict]:
    logits = np.ascontiguousarray(logits, dtype=np.float32)
    t32 = np.asarray(targets).astype(np.float32)  # values are 0/1; lossless
    in_maps = []
    for k in range(NCORES):
        sl = slice(k * SHARD, (k + 1) * SHARD)
        xk = np.empty((P, 2 * F), np.float32)
        xk[:, 0:F] = logits[sl].reshape(P, F)
        xk[:, F : 2 * F] = t32[sl].reshape(P, F)
        in_maps.append({"x": xk})
    return in_maps


def combine(outs: np.ndarray) -> np.ndarray:
    """All-reduce the [NCORES, P, 5] partials and apply the closed form."""
    tot = outs.astype(np.float64).sum(axis=(0, 1))
    g1, g2, c, m1, m2 = tot
    n_pos = c
    n_neg = float(N) - c
    sp1 = c - m1
    sp2 = c - 2.0 * m1 + m2
    sn1 = g1 - m1
    sn2 = g2 - m2
    loss = (n_neg * sp2 + 2.0 * sp1 * sn1 + n_pos * sn2) / (n_pos * n_neg)
    return np.array(loss, dtype=np.float32)


def kernel(logits: np.ndarray, targets: np.ndarray, **run_kwargs):
    nc = _get_nc()
    res = bass_utils.run_bass_kernel_spmd(
        nc, make_in_maps(logits, targets), core_ids=list(range(NCORES)), **run_kwargs
    )
    outs = np.stack([r["o"][:, 0:5] for r in res.results])  # [8, 128, 5]
    out = combine(outs)
    _CACHE["last_results"] = res
    return out
